# revision 6
# baseline (speedup 1.0000x reference)
"""Eval-mode ClassConditionalBatchNorm2d on 8 Trainium2 NeuronCores.

Math: for each sample b with label l:
    use_class = (alpha > 0) & (class_counts[l] >= 100)
    mean/var  = blend of (global, class[l]) stats if use_class else global
    out       = (x - mean) / sqrt(var + eps) * weight + bias

This folds to a per-(sample, channel) affine:  out = x * scale + shift with
    scale[b,c] = weight[c] / sqrt(var[b,c] + eps)
    shift[b,c] = bias[c] - mean[b,c] * scale[b,c]

The [B=64, C=256] scale/shift tables are tiny and computed on host in f32.
The device kernel streams x through SBUF applying one fused DVE
tensor_scalar (mult+add, per-partition f32 scalars) per (sample, channel
half) — pure HBM-bound streaming.

Datapath is fp16: the x shards are cast to f16 on host, the kernel
reads/writes f16 (halving HBM traffic vs f32 — per core 2 x 12.85 MB at
~358 GB/s => ~72 us roofline), and the result is cast back to f32 on host.
f16 keeps 11 mantissa bits so the end-to-end error is ~1e-3 absolute
(~1e-4 scale-relative), far inside the 2e-2 gate. The affine itself is
computed by the DVE in f32 from the f32 scalar table; only x quantization
and the final f16 store round.

Layout: channels 2p and 2p+1 sit on partition p ("(p h)" split), so each
partition's DMA line for one sample is a single contiguous 12544-byte DRAM
chunk (f16), keeping descriptors large. Tiles hold `fuse_halves` samples
([128, G*2, 3136] f16); in/out pools pipeline load/compute/store.

Sharding: pure data parallel over batch — 8 samples per core, the table
shard is per-core, cores never communicate.
"""

import numpy as np
from contextlib import ExitStack

B, C, H, W = 64, 256, 56, 56
HW = H * W
N_CORES = 8
BPC = B // N_CORES  # samples per core
N_HALF = C // 128   # channel halves per partition (2)
EPS = 1e-5
MIN_COUNT = 100.0

# Device-kernel configuration (must match between kernel() and bench).
CONFIG = dict(bufs=4, obufs=3, fuse_halves=2, fp16=True, layout="ph",
              in_place=False, store_eng="sync")

_PROGRAM_CACHE = {}
LAST_RESULTS = None  # BassKernelResults of the most recent run (for profiling)


def _build_program(iters=1, bufs=4, dyn_loop=None, in_place=False,
                   fuse_halves=2, split=1, obufs=None, store_eng="sync",
                   fp16=True, layout="ph"):
    """Build + compile the single-core SPMD Bass program (cached).

    iters > 1 repeats the identical sweep back-to-back inside one NEFF;
    dyn_loop=N wraps the sweep in a hardware For loop of N trips. Both are
    used only by the benchmark harness to measure per-sweep cost.
    in_place applies the affine into the input tile (one pool, more bufs).
    fuse_halves=G >= 1 loads/stores G whole samples (both channel halves)
    per DMA. layout="ph" puts channels (2p, 2p+1) on partition p
    (contiguous DRAM line per sample); "hp" puts (p, 128+p) there.
    """
    fuse_halves = int(fuse_halves)
    obufs = bufs if obufs is None else obufs
    key = (iters, bufs, dyn_loop, in_place, fuse_halves, split, obufs,
           store_eng, fp16, layout)
    if key in _PROGRAM_CACHE:
        return _PROGRAM_CACHE[key]

    import concourse.tile as tile
    from concourse import bacc, mybir

    f32 = mybir.dt.float32
    dt_x = mybir.dt.float16 if fp16 else f32
    nc = bacc.Bacc(
        "TRN2", target_bir_lowering=False, debug=False, num_devices=N_CORES
    )
    x_ap = nc.dram_tensor("x", [BPC, C, HW], dt_x, kind="ExternalInput").ap()
    tab_ap = nc.dram_tensor(
        "tables", [128, BPC * N_HALF * 2], f32, kind="ExternalInput"
    ).ap()
    out_ap = nc.dram_tensor("out", [BPC, C, HW], dt_x, kind="ExternalOutput").ap()

    pat = "g (p h) f -> p g h f" if layout == "ph" else "g (h p) f -> p g h f"

    with tile.TileContext(nc) as tc:
        with ExitStack() as ctx:
            tabp = ctx.enter_context(tc.tile_pool(name="tab", bufs=1))
            xp = ctx.enter_context(tc.tile_pool(name="xs", bufs=bufs))
            outp = None if in_place else ctx.enter_context(
                tc.tile_pool(name="os", bufs=obufs)
            )
            st_eng = {"sync": nc.sync, "scalar": nc.scalar,
                      "gpsimd": nc.gpsimd}[store_eng]

            tab = tabp.tile([128, BPC * N_HALF * 2], f32)
            nc.sync.dma_start(tab[:], tab_ap[:])

            def sweep():
                G = fuse_halves  # samples per tile
                for b0 in range(0, BPC, G):
                    t = xp.tile([128, G, N_HALF, HW], dt_x)
                    src = x_ap[b0 : b0 + G].rearrange(pat, h=N_HALF)
                    nc.sync.dma_start(t[:], src)
                    o = t if in_place else outp.tile([128, G, N_HALF, HW], dt_x)
                    for g in range(G):
                        for h in range(N_HALF):
                            r = N_HALF * (b0 + g) + h
                            nc.vector.tensor_scalar(
                                o[:, g, h, :],
                                t[:, g, h, :],
                                tab[:, 2 * r : 2 * r + 1],
                                tab[:, 2 * r + 1 : 2 * r + 2],
                                mybir.AluOpType.mult,
                                mybir.AluOpType.add,
                            )
                    dst = out_ap[b0 : b0 + G].rearrange(pat, h=N_HALF)
                    st_eng.dma_start(dst, o[:])

            if dyn_loop is not None:
                with tc.For_i(0, dyn_loop, 1):
                    for _ in range(iters):
                        sweep()
            else:
                for _ in range(iters):
                    sweep()

    nc.compile()
    _PROGRAM_CACHE[key] = nc
    return nc


def _scale_shift(labels, weight, bias, global_mean, global_var,
                 class_mean, class_var, class_counts, alpha):
    """Per-sample affine tables [B, C], mirroring the reference's f32 branch
    selection exactly; the weight/sqrt fold is done in f64 for accuracy."""
    labels = np.asarray(labels).astype(np.int64).reshape(-1)
    a = np.float32(np.asarray(alpha).reshape(()))
    one_m_a = np.float32(1.0) - a

    use_class = (float(a) > 0.0) & (
        np.asarray(class_counts, np.float32)[labels] >= np.float32(MIN_COUNT)
    )  # [B]
    gm = np.asarray(global_mean, np.float32)
    gv = np.asarray(global_var, np.float32)
    blend_mean = one_m_a * gm[None, :] + a * np.asarray(class_mean, np.float32)[labels]
    blend_var = np.clip(
        one_m_a * gv[None, :] + a * np.asarray(class_var, np.float32)[labels],
        np.float32(EPS),
        None,
    )
    mean = np.where(use_class[:, None], blend_mean, gm[None, :])  # [B, C] f32
    var = np.where(use_class[:, None], blend_var, gv[None, :])

    scale64 = np.asarray(weight, np.float64)[None, :] / np.sqrt(
        var.astype(np.float64) + np.float64(EPS)
    )
    shift64 = np.asarray(bias, np.float64)[None, :] - mean.astype(np.float64) * scale64
    return scale64.astype(np.float32), shift64.astype(np.float32)


def make_in_maps(inputs, fp16=None, layout=None):
    """Per-core input maps (x shard + affine table) from the full input dict."""
    fp16 = CONFIG["fp16"] if fp16 is None else fp16
    layout = CONFIG["layout"] if layout is None else layout
    x = np.asarray(inputs["x"], np.float32)
    if fp16:
        x = x.astype(np.float16)
    scale, shift = _scale_shift(
        inputs["labels"], inputs["weight"], inputs["bias"],
        inputs["global_mean"], inputs["global_var"], inputs["class_mean"],
        inputs["class_var"], inputs["class_counts"], inputs["alpha"],
    )
    in_maps = []
    for c in range(N_CORES):
        xs = x[c * BPC : (c + 1) * BPC].reshape(BPC, C, HW)
        if layout == "ph":
            # col 4b+2h+k on partition p holds (scale, shift)[b, 2p+h]
            sc = scale[c * BPC : (c + 1) * BPC].reshape(BPC, 128, N_HALF)
            sh = shift[c * BPC : (c + 1) * BPC].reshape(BPC, 128, N_HALF)
            st = np.stack([sc, sh], axis=-1)  # [b, p, h, k]
            tab = np.ascontiguousarray(
                st.transpose(1, 0, 2, 3).reshape(128, BPC * N_HALF * 2)
            )
        else:
            # col 4b+2h+k on partition p holds (scale, shift)[b, h*128+p]
            sc = scale[c * BPC : (c + 1) * BPC].reshape(BPC, N_HALF, 128)
            sh = shift[c * BPC : (c + 1) * BPC].reshape(BPC, N_HALF, 128)
            st = np.stack([sc, sh], axis=-1)  # [b, h, p, k]
            tab = np.ascontiguousarray(
                st.transpose(2, 0, 1, 3).reshape(128, BPC * N_HALF * 2)
            )
        in_maps.append({"x": np.ascontiguousarray(xs), "tables": tab})
    return in_maps


def kernel(x, labels, weight, bias, global_mean, global_var,
           class_mean, class_var, class_counts, alpha):
    global LAST_RESULTS
    from concourse.bass_utils import run_bass_kernel_spmd

    nc = _build_program(**CONFIG)
    in_maps = make_in_maps({
        "x": x, "labels": labels, "weight": weight, "bias": bias,
        "global_mean": global_mean, "global_var": global_var,
        "class_mean": class_mean, "class_var": class_var,
        "class_counts": class_counts, "alpha": alpha,
    })

    res = run_bass_kernel_spmd(nc, in_maps, list(range(N_CORES)))
    LAST_RESULTS = res

    out = np.empty((B, C, H, W), np.float32)
    for c in range(N_CORES):
        out[c * BPC : (c + 1) * BPC] = np.asarray(
            res.results[c]["out"], np.float32
        ).reshape(BPC, C, H, W)
    return out
